# revision 5
# baseline (speedup 1.0000x reference)
"""EuclideanDeconf kernel for 8x TRN2 NeuronCores (v3).

Computes out[b, c] = (2/D) * x @ W.T - ||x||^2/D - ||W||^2/D
for x [16384, 1024] f32, W [2048, 1024] f32 -> out [16384, 2048] f32.

Sharding: data-parallel over batch. Each of the 8 cores gets 2048 rows of
x and the full W. Host does layout-only work (transpose / cast / shard /
concat); all FLOPs (matmul, norms, combine) run on device.

HBM traffic 26MB -> 16MB/core vs the 124us v1: x and W shipped pre-cast
e4m3 for the matmuls (p-major layouts so every DMA is >=2KB/partition
descriptors) plus compact row-major copies for the norm reductions
(x bf16, W fp8); y stored bf16 and upcast on host.

Per-core engine plan (PE is the 56us floor; everything else hides under):
  PE:     24 warmup bf16 matmuls (HAM un-throttle) + 256 e4m3 DoubleRow
          matmuls (~220ns each when fed) - nothing else
  DVE:    w2: 16 scalar_tensor_tensor reduces (fp8 wrd; accum = -sum/256D)
          x2: b-tiles 0-9 same-form reduces from bf16 xrd
          pass2 for b-tiles 4-15 (y = t + (-w2rep), bf16 2x rate)
  ACT:    x2 for b-tiles 10-15 (Square activation + accum_out; positive, so
          those b-tiles run pass1 with negated scale and pass2 subtracts)
          pass1 x32: t = cross_scale*psum - x2[b] (Identity, bias AP)
          xrd/wrd load issues on the ACT HWDGE ring (ring2)
  GPSIMD: pass2 for b-tiles 0-3 (tensor_tensor add; covers the window
          where DVE is still busy with w2)
  DMA:    ring1 (SP): x fp8 chunks, W fp8 cj-blocks, w2 row gather +
          stride-0 broadcast, y-out; ring2 (ACT): xrd + wrd
Engine queues are strict FIFO, so DMA issues that wait on late producers
(y-out on pass2) live on SP, never on ACT where they would block pass1.

Numerics: cross term e4m3 (x plain, W prescaled by 16; the epilogue scale
folds it out), x2 from bf16 x reduced in f32, w2 from fp8 16W, y and the
pass1 intermediate bf16 -> norm rel err ~2.6e-3 (gate 2e-2), dominated by
the two bf16 roundings of the ~1.0-magnitude output.
"""

import numpy as np
import ml_dtypes

# Problem constants (hardcoded; kernel.py must be self-contained).
B, D, C = 16384, 1024, 2048
NCORES = 8
BSH = B // NCORES   # 2048 rows of x per core
P = 128             # partitions
KT = D // P         # 8 contraction k-planes
NB = BSH // P       # 16 b-tiles per core
NCH = 4             # x chunks (512 b-cols each)
BCH = BSH // NCH    # 512
NCJ = 4             # W column blocks (512 classes each)
CJW = C // NCJ      # 512

N_X2_DVE = 10       # b-tiles 0..9 reduce x2 on DVE; 10..15 on ACT
N_P2_GP = 4         # b-tiles 0..3 run pass2 on gpsimd

_CACHE = {}


def _build_nc():
    import concourse.tile as tile
    import concourse.mybir as mybir
    import concourse.bass as bass
    from concourse.ap import AP
    from concourse import bacc

    f32 = mybir.dt.float32
    bf16 = mybir.dt.bfloat16
    fp8 = mybir.dt.float8e4
    PSUM = bass.MemorySpace.PSUM
    Identity = mybir.ActivationFunctionType.Identity
    Square = mybir.ActivationFunctionType.Square
    MULT = mybir.AluOpType.mult
    ADD = mybir.AluOpType.add
    SUB = mybir.AluOpType.subtract
    DR = mybir.MatmulPerfMode.DoubleRow

    # W is host-prescaled by 16 (keeps e4m3 out of subnormals); the
    # epilogue scale folds the 1/16 back out.
    cross_scale = 2.0 / D / 16.0

    nc = bacc.Bacc(
        "TRN2",
        target_bir_lowering=False,
        debug=False,
        enable_asserts=False,
    )
    # p-major host layouts (see _prep_inputs)
    xTb = nc.dram_tensor("xTb", [NCH, P, KT * BCH], fp8, kind="ExternalInput").ap()
    wTb = nc.dram_tensor("wTb", [NCJ, P, KT * CJW], fp8, kind="ExternalInput").ap()
    xRd = nc.dram_tensor("xRd", [P, NB * D], bf16, kind="ExternalInput").ap()
    wRd = nc.dram_tensor("wRd", [P, (C // P) * D], fp8, kind="ExternalInput").ap()
    y = nc.dram_tensor("y", [BSH, C], bf16, kind="ExternalOutput").ap()

    with tile.TileContext(nc) as tc:
        with (
            tc.tile_pool(name="consts", bufs=1) as cpool,
            tc.tile_pool(name="wpool", bufs=1) as wpool,
            tc.tile_pool(name="xpool", bufs=1) as xpool,
            tc.tile_pool(name="rpool", bufs=1) as rpool,
            tc.tile_pool(name="sdve", bufs=1) as sdve,
            tc.tile_pool(name="sact", bufs=1) as sact,
            tc.tile_pool(name="tpool", bufs=10) as tpool,
            tc.tile_pool(name="ypool", bufs=3) as ypool,
            tc.tile_pool(name="pmain", bufs=3, space=PSUM) as pmain,
            tc.tile_pool(name="pwarm", bufs=1, space=PSUM) as pwarm,
        ):
            # ---- consts + PE warmup (covers the DMA ramp, holds HAM) ----
            warmz = cpool.tile([P, 512], bf16)
            nc.gpsimd.memset(warmz[:], 0.0)
            warm_ps = pwarm.tile([P, 512], f32, tag="wps", bufs=1)
            for _ in range(24):
                nc.tensor.matmul(warm_ps[:], warmz[:, 0:P], warmz[:],
                                 start=True, stop=True)
            # touch ACT early so its function-table DMA is off the
            # critical path by the time the first pass1 runs
            warm1 = cpool.tile([1, 1], f32)
            nc.scalar.activation(warm1[:], warmz[0:1, 0:1], Identity,
                                 bias=0.0, scale=1.0)

            # ---- SBUF tiles ----
            wcj = [wpool.tile([P, KT, CJW], fp8, name=f"wcj{c}")
                   for c in range(NCJ)]
            xch = [xpool.tile([P, KT, BCH], fp8, name=f"xch{c}")
                   for c in range(NCH)]
            xrd = rpool.tile([P, NB, D], bf16)
            wrd = rpool.tile([P, C // P, D], fp8)
            x2cols = rpool.tile([P, NB], f32)
            w2cf = rpool.tile([P, C // P], f32)
            w2cb = rpool.tile([P, C // P], bf16)
            w2row = rpool.tile([1, C], bf16)
            w2rep = rpool.tile([P, C], bf16)

            # ---- ring1 (SP): matmul operands, ordered for first use ----
            nc.sync.dma_start(
                xch[0][:], xTb[0].rearrange("p (k b) -> p k b", k=KT))
            for cj in range(NCJ):
                nc.sync.dma_start(
                    wcj[cj][:], wTb[cj].rearrange("p (k b) -> p k b", k=KT))
            for ch in range(1, NCH):
                nc.sync.dma_start(
                    xch[ch][:], xTb[ch].rearrange("p (k b) -> p k b", k=KT))

            # ---- ring2 (ACT): reduction operands ----
            def dma_xrd(lo, hi):
                src = xRd[:, lo * D:hi * D]
                nc.scalar.dma_start(
                    xrd[:, lo:hi, :],
                    src.rearrange("p (j d) -> p j d", j=hi - lo))

            dma_xrd(0, 4)
            for piece in range(2):
                src = wRd[:, piece * 8 * D:(piece + 1) * 8 * D]
                nc.scalar.dma_start(
                    wrd[:, 8 * piece:8 * piece + 8, :],
                    src.rearrange("p (j d) -> p j d", j=8))
            dma_xrd(4, 10)
            dma_xrd(10, 16)

            # ---- norm reductions ----
            # DVE form: accum_out = sum((v * -s) * v) = -||row||^2 * s
            def x2_dve(j):
                scr = sdve.tile([P, D], bf16, tag="scr", name=f"sx{j}")
                nc.vector.scalar_tensor_tensor(
                    out=scr[:], in0=xrd[:, j, :], scalar=-1.0 / D,
                    in1=xrd[:, j, :], op0=MULT, op1=MULT,
                    accum_out=x2cols[:, j:j + 1],
                )

            def w2_dve(t):
                scr = sdve.tile([P, D], bf16, tag="scr", name=f"sw{t}")
                nc.vector.scalar_tensor_tensor(
                    out=scr[:], in0=wrd[:, t, :], scalar=-1.0 / (256.0 * D),
                    in1=wrd[:, t, :], op0=MULT, op1=MULT,
                    accum_out=w2cf[:, t:t + 1],
                )

            # ACT form (positive): accum_out = sum(Square(v/32)) = ||v||^2/D
            def x2_act(j):
                scr = sact.tile([P, D], bf16, tag="scr", name=f"sa{j}")
                nc.scalar.activation(scr[:], xrd[:, j, :], Square,
                                     bias=0.0, scale=1.0 / 32.0,
                                     accum_out=x2cols[:, j:j + 1])

            # DVE emission: x2 pieces interleaved so x2[j] lands before
            # pass1(j) (~3.5us per b-tile); w2 fills the gaps and finishes
            # ~35us, after which pass2 catches up from the t backlog.
            for j in range(4):
                x2_dve(j)
            for t in range(8):
                w2_dve(t)
            x2_dve(4)
            x2_dve(5)
            for t in range(8, 12):
                w2_dve(t)
            x2_dve(6)
            x2_dve(7)
            for t in range(12, 16):
                w2_dve(t)
            x2_dve(8)
            x2_dve(9)
            nc.vector.tensor_copy(w2cb[:], w2cf[:])

            # ---- w2rep = -w2 broadcast [P, C], all on ring1 (SP) ----
            # gather [128, 16] partition-major -> [1, C] row (p-outer order
            # matches wRd's reshape(P, C//P, D) class layout)
            nc.sync.dma_start(w2row[:], w2cb[:])
            # replicate partition 0 to all 128 via a stride-0 free dim
            wr = w2row[:]
            wr_b = AP(wr.tensor, wr.offset,
                      [list(wr.ap[0]), [0, P], list(wr.ap[1])])
            nc.sync.dma_start(w2rep[:], wr_b)

            # ---- main loop: 16 b-tiles of 16 DR matmuls + epilogue ----
            for j in range(NB):
                ch, jl = divmod(j, NB // NCH)
                act_form = j >= N_X2_DVE   # x2[j] positive (ACT Square)
                t_t = tpool.tile([P, C], bf16, tag="t", name=f"t{j}")
                for h in range(2):
                    ps = pmain.tile([P, 1024], f32, tag="ps", name=f"ps{j}_{h}")
                    for cj in (2 * h, 2 * h + 1):
                        for k2 in range(KT // 2):
                            nc.tensor.matmul(
                                ps[:, (cj % 2) * 512:(cj % 2) * 512 + 512],
                                xch[ch][:, 2 * k2:2 * k2 + 2,
                                        jl * P:(jl + 1) * P],
                                wcj[cj][:, 2 * k2:2 * k2 + 2, :],
                                start=(k2 == 0),
                                stop=(k2 == KT // 2 - 1),
                                perf_mode=DR,
                            )
                    # pass1: t = cross_scale*psum - x2[b]
                    #   (ACT-form x2 is positive: t' = x2 - cross instead,
                    #    and pass2 subtracts)
                    nc.scalar.activation(
                        t_t[:, h * 1024:(h + 1) * 1024], ps[:], Identity,
                        bias=x2cols[:, j:j + 1],
                        scale=(-cross_scale if act_form else cross_scale))
                # emit ACT x2 squares ~2 b-tiles ahead of their pass1
                if 8 <= j < 8 + (NB - N_X2_DVE):
                    x2_act(j + 2)

                def pass2(dst, a_t, w_slice, eng):
                    if act_form:
                        eng.tensor_tensor(dst, w_slice, a_t, op=SUB)
                    else:
                        eng.tensor_tensor(dst, a_t, w_slice, op=ADD)

                if j == NB - 1:
                    # last b-tile: per-half pass2 + store, shortest tail
                    for h in range(2):
                        sl = slice(h * 1024, (h + 1) * 1024)
                        yh = ypool.tile([P, 1024], bf16, tag="yh",
                                        name=f"yh{h}")
                        pass2(yh[:], t_t[:, sl], w2rep[:, sl], nc.vector)
                        nc.sync.dma_start(y[j * P:(j + 1) * P, sl], yh[:])
                else:
                    eng = nc.gpsimd if j < N_P2_GP else nc.vector
                    yt = ypool.tile([P, C], bf16, tag="yt", name=f"yt{j}")
                    pass2(yt[:], t_t[:], w2rep[:], eng)
                    nc.sync.dma_start(y[j * P:(j + 1) * P, :], yt[:])

    nc.compile()
    return nc


def _get_nc():
    if "nc" not in _CACHE:
        _CACHE["nc"] = _build_nc()
    return _CACHE["nc"]


def _prep_inputs(x, W):
    x = np.ascontiguousarray(x, dtype=np.float32)
    W = np.ascontiguousarray(W, dtype=np.float32)
    e4 = ml_dtypes.float8_e4m3
    bf = ml_dtypes.bfloat16

    W16 = W * np.float32(16.0)
    # W fp8 cj-blocks, p-major: wTb[cj, p, k*512 + b] = 16*W.T[k*128+p,
    # cj*512+b]
    wTb = np.ascontiguousarray(
        W16.T.astype(e4).reshape(KT, P, NCJ, CJW).transpose(2, 1, 0, 3)
    ).reshape(NCJ, P, KT * CJW)
    # W fp8 rows, p-major by (p, t): partition p holds classes p*16+t
    wRd = np.ascontiguousarray(
        W16.astype(e4).reshape(P, C // P, D)
    ).reshape(P, (C // P) * D)

    in_maps = []
    for i in range(NCORES):
        xs = x[i * BSH:(i + 1) * BSH, :]                # [2048, 1024]
        xT8 = xs.T.astype(e4)                           # [D, BSH]
        xTbi = np.ascontiguousarray(
            xT8.reshape(KT, P, NCH, BCH).transpose(2, 1, 0, 3)
        ).reshape(NCH, P, KT * BCH)
        xRdi = np.ascontiguousarray(
            xs.astype(bf).reshape(NB, P, D).transpose(1, 0, 2)
        ).reshape(P, NB * D)
        in_maps.append({"xTb": xTbi, "xRd": xRdi, "wTb": wTb, "wRd": wRd})
    return in_maps


def run(x, W, trace=False, **trace_kwargs):
    """Run on the 8 cores; returns (out [B, C] f32, BassKernelResults)."""
    from concourse import bass_utils

    nc = _get_nc()
    in_maps = _prep_inputs(x, W)
    res = bass_utils.run_bass_kernel_spmd(
        nc, in_maps, core_ids=list(range(NCORES)), trace=trace, **trace_kwargs
    )
    out = np.concatenate(
        [r["y"].astype(np.float32) for r in res.results], axis=0)
    return out, res


def kernel(x, W, task_id=None, **_unused):
    out, _ = run(np.asarray(x), np.asarray(W), trace=False)
    return out
